# revision 24
# baseline (speedup 1.0000x reference)
"""Polynomial features (degree 2) + linear layer, distributed over 8 TRN2 cores.

reference: A = [x, {x_i*x_j for i<=j}] (8384 coeffs); out = A @ W.T + b.

Device algorithm (per core, batch shard 4096, feature-on-partition layout):
  - pairs are enumerated by circular distance class s in 0..64:
      class s, lane p  ->  unordered pair {p, (p+s) % 128}
    (each unordered pair appears exactly once; s=64 lanes >=64 are dups
    with zeroed weights)
  - host ships 16 rotated copies of x^T (rot d: row p = feature (p+d)%128)
    for d in D = {0..8, 16, 24, 32, 40, 48, 56, 64}; every class s is one
    bf16 DVE tensor_mul of two rotations with b - a = s (the hardware only
    allows 32-aligned partition bases, so all ops are full 128-partition,
    base 0 - the rotations do the shifting)
  - 66 matmuls (1 linear chunk + 65 class chunks, K=128 each) accumulate
    into PSUM [64 outs, 512 batch]; W is permuted host-side to match;
    bias is added in the PSUM->SBUF copy (DVE tensor_scalar_add)
  - TPB instructions have a single sync-wait slot, but Tile emits multiple
    waits on slot-recycling instructions; _split_multiwaits() post-processes
    the BIR, hoisting extra waits onto injected same-engine NOPs
"""

import numpy as np
import ml_dtypes

INPUT_DIM = 128
OUTPUT_DIM = 64
BATCH = 32768
N_CORES = 8
B_CORE = BATCH // N_CORES  # 4096
TILE_B = 512
N_TILES = B_CORE // TILE_B  # 8

ROT_SET = [0, 1, 2, 3, 4, 5, 6, 7, 8, 16, 24, 32, 40, 48, 56, 64]
N_ROT = len(ROT_SET)
ROT_IDX = {d: i for i, d in enumerate(ROT_SET)}

import os

GPS_OP_IDS = tuple(
    int(v) for v in os.environ.get("K_GPS_OPS", "").split(",") if v != ""
)


def _class_ops():
    """(a, b) rotation pair per distance class s=0..64 with b - a = s."""
    ops = []
    for s in range(65):
        if s <= 8:
            a, b = 0, s
        else:
            k = (s - 1) // 8  # 1..7
            anchor = 8 * k + 8
            a, b = anchor - s, anchor
        assert a in ROT_SET and b in ROT_SET and b - a == s, (s, a, b)
        ops.append((a, b))
    return ops


CLASS_OPS = _class_ops()


def _build_device_weights(W, b):
    """Permute W [64, 8384] into the device K-block layout.

    Returns w_packed [128, 66*64]: block j (j=0 linear, j=1+s class s)
    lives at free columns [j*64, (j+1)*64), partition p = K row p.
    Class s row p -> pair {p, (p+s)%128}; s=64 rows p>=64 are zeroed dups.
    """
    W = np.asarray(W, np.float32)
    n = INPUT_DIM
    pair_off = {}
    c = 0
    for i in range(n):
        for j in range(i, n):
            pair_off[(i, j)] = c
            c += 1
    assert c == 8256

    Wd = np.zeros((66, 128, OUTPUT_DIM), np.float32)
    Wd[0] = W[:, 0:128].T  # linear block
    seen = set()
    for s in range(65):
        a, _bb = CLASS_OPS[s]
        for p in range(128):
            u = (p + a) % 128
            v = (p + a + s) % 128
            i, j = (u, v) if u <= v else (v, u)
            if (i, j) in seen:
                continue  # duplicate lane (s=64 second half)
            seen.add((i, j))
            Wd[1 + s, p] = W[:, 128 + pair_off[(i, j)]]
    assert len(seen) == 8256, len(seen)
    w_packed = np.ascontiguousarray(
        Wd.transpose(1, 0, 2).reshape(128, 66 * OUTPUT_DIM)
    ).astype(ml_dtypes.bfloat16)
    return w_packed, np.asarray(b, np.float32)


def _split_multiwaits(nc, mybir):
    """TPB instructions have one sync-wait slot; hoist extras onto NOPs."""
    import bass_rust

    n_split = 0
    for fn in nc.m.functions:
        for bb in fn.blocks:
            out = []
            changed = False
            for inst in bb.instructions:
                si = getattr(inst, "sync_info", None)
                if si is not None and si.on_wait and len(si.on_wait) > 1:
                    for w in si.on_wait[:-1]:
                        n_split += 1
                        nop = bass_rust.InstNoOp(
                            name=f"I-mw{n_split}",
                            engine=inst.engine,
                            ins=[],
                            outs=[],
                            sync_info=mybir.SyncInfo(on_wait=[w], on_update=[]),
                            bass_nofuse=True,
                        )
                        out.append(nop)
                    inst.sync_info = mybir.SyncInfo(
                        on_wait=[si.on_wait[-1]], on_update=si.on_update
                    )
                    changed = True
                out.append(inst)
            if changed:
                bb.instructions = out
    return n_split


def build(x, W, b):
    """Build the Bass graph and per-core input maps. Returns (nc, in_maps)."""
    import concourse.bass as bass
    import concourse.mybir as mybir
    from concourse import tile

    bf16 = mybir.dt.bfloat16
    f32 = mybir.dt.float32

    # ---- host preprocessing ----
    xT = np.ascontiguousarray(np.asarray(x, np.float32).T).astype(
        ml_dtypes.bfloat16
    )  # [128, 32768]
    # xall[p, i, n] = feature (p + ROT_SET[i]) % 128 of sample n
    xall = np.stack([np.roll(xT, -d, axis=0) for d in ROT_SET], axis=1)
    w_packed, bias = _build_device_weights(W, b)

    # ---- device graph ----
    nc = bass.Bass()
    x_in = nc.declare_dram_parameter(
        "xall", [N_TILES, 128, N_ROT, TILE_B], bf16, isOutput=False
    )
    w_in = nc.declare_dram_parameter("Wd", [128, 66 * 64], bf16, isOutput=False)
    b_in = nc.declare_dram_parameter("bias", [OUTPUT_DIM, 1], f32, isOutput=False)
    out_ext = nc.declare_dram_parameter(
        "outT", [OUTPUT_DIM, B_CORE], f32, isOutput=True
    )

    # multi-class ops: op 0 = class 0 alone; ops 1..16 = quads (4k+1..4k+4),
    # each quad having constant-stride rotation indices
    MC_OPS = [[0]] + [[4 * k + 1, 4 * k + 2, 4 * k + 3, 4 * k + 4] for k in range(16)]
    GPS_OPS = set(GPS_OP_IDS)  # op indices computed on GpSimd

    def rot_group_ap(xrt, classes):
        """[128, len(classes), TILE_B] APs (in0, in1)."""
        m = len(classes)
        us = [ROT_IDX[CLASS_OPS[s][0]] for s in classes]
        vs = [ROT_IDX[CLASS_OPS[s][1]] for s in classes]

        def mk(idx):
            if all(i == idx[0] for i in idx):
                return xrt[:, idx[0] : idx[0] + 1, :].to_broadcast(
                    [128, m, TILE_B]
                )
            d = idx[1] - idx[0]
            assert all(idx[j + 1] - idx[j] == d for j in range(m - 1)), idx
            return xrt[:, idx[0] :: d, :][:, 0:m, :]

        return mk(us), mk(vs)

    with tile.TileContext(nc) as tc:
        with (
            tc.tile_pool(name="consts", bufs=1) as consts,
            tc.tile_pool(name="xc", bufs=3) as xcp,
            tc.tile_pool(name="lin", bufs=3) as linp,
            tc.tile_pool(name="prod", bufs=12) as prodp,
            tc.tile_pool(name="prodg", bufs=5) as prodgp,
            tc.tile_pool(name="outp", bufs=3) as outp,
            tc.tile_pool(name="psum", bufs=2, space="PSUM") as psump,
        ):
            w_sb = consts.tile([128, 66 * 64], bf16)
            nc.sync.dma_start(w_sb[:], w_in[:])
            b_sb = consts.tile([OUTPUT_DIM, 1], f32)
            nc.sync.dma_start(b_sb[:], b_in[:])

            xc_tiles = [None] * (N_TILES + 2)

            def load_xc(t):
                if t >= N_TILES:
                    return
                xt = xcp.tile([128, N_ROT, TILE_B], bf16, tag="xc", name="xc_t")
                nc.sync.dma_start(xt[:], x_in[t][:])
                xc_tiles[t] = xt

            load_xc(0)
            load_xc(1)
            for t in range(N_TILES):
                load_xc(t + 2)
                xrt = xc_tiles[t]
                # DVE copy of rotation 0: matmul operand (keeps xc DVE-only)
                lin_t = linp.tile([128, TILE_B], bf16, tag="lin", name="lin_t")
                nc.vector.tensor_copy(lin_t[:], xrt[:, 0, :])

                # acc halves: even classes + linear -> partitions 0:64
                # (array cols 0-63), odd classes -> partitions 64:128
                acc = psump.tile([128, TILE_B], f32, name="acc")
                nc.tensor.matmul(
                    acc[0:64, :],
                    w_sb[:, 0:64],
                    lin_t[:],
                    start=True,
                    stop=False,
                    tile_position=(0, 0),
                )
                first_odd = True
                for k, classes in enumerate(MC_OPS):
                    m = len(classes)
                    pool_k = prodgp if k in GPS_OPS else prodp
                    tag = ("prodg" if k in GPS_OPS else "prod") + str(m)
                    p_t = pool_k.tile(
                        [128, m, TILE_B], bf16, tag=tag, name="p_t"
                    )
                    in0, in1 = rot_group_ap(xrt, classes)
                    eng = nc.gpsimd if k in GPS_OPS else nc.vector
                    eng.tensor_mul(p_t[:], in0, in1)
                    views = [
                        (s, p_t[:, j, :]) for j, s in enumerate(classes)
                    ]
                    for s, rhs in views:
                        half = s % 2
                        blk = 1 + s
                        is_last_even = s == 64
                        is_last_odd = s == 63
                        nc.tensor.matmul(
                            acc[64 * half : 64 * half + 64, :],
                            w_sb[:, blk * 64 : (blk + 1) * 64],
                            rhs,
                            start=(half == 1 and first_odd),
                            stop=(is_last_even or is_last_odd),
                            tile_position=(0, 64 * half),
                        )
                        if half == 1:
                            first_odd = False

                o_t = outp.tile([OUTPUT_DIM, TILE_B], f32, tag="o", name="o_t")
                # ACT: bias + even-half from PSUM; DVE: add odd-half
                nc.scalar.activation(
                    o_t[:],
                    acc[0:64, :],
                    mybir.ActivationFunctionType.Identity,
                    bias=b_sb[:, 0:1],
                )
                nc.vector.tensor_add(o_t[:], o_t[:], acc[64:128, :])
                nc.sync.dma_start(
                    out_ext[:, t * TILE_B : (t + 1) * TILE_B], o_t[:]
                )

    _split_multiwaits(nc, mybir)

    # ---- per-core input maps ----
    in_maps = []
    for c in range(N_CORES):
        cs = xall[:, :, c * B_CORE : (c + 1) * B_CORE]  # [128, 16, 4096]
        xtiles = np.ascontiguousarray(
            cs.reshape(128, N_ROT, N_TILES, TILE_B).transpose(2, 0, 1, 3)
        )  # [N_TILES, 128, 16, TILE_B]
        in_maps.append(
            {
                "xall": xtiles,
                "Wd": w_packed,
                "bias": bias.reshape(OUTPUT_DIM, 1),
            }
        )
    return nc, in_maps


def kernel(x, W, b, indices_0, indices_1):
    from concourse.bass_utils import run_bass_kernel_spmd

    nc, in_maps = build(x, W, b)
    res = run_bass_kernel_spmd(nc, in_maps, list(range(N_CORES))).results
    out = np.concatenate([np.asarray(r["outT"], np.float32).T for r in res], axis=0)
    return out
